# revision 1
# baseline (speedup 1.0000x reference)
"""BiMamba2D (VMamba-style 4-direction selective scan) Trainium2 Bass kernel.

Sharding: 8 cores = 4 batches x 2 scan layouts (hw / wh).  The wh layout is
realized by host-transposing the input image (and swapping the conv kernel's
spatial taps), so every core runs the same SPMD program.  Each core computes
both time directions (forward + reversed APs) of its layout and emits a
partial (L, 96) output; the host sums partials (gating and the output
projection are linear across the four direction contributions).

Scan-state layout: d-blocks of 8 channels x 16 states = 128 partitions
(row p of a d-block tile holds channel db*8 + p//16, state p%16).  The
recurrence runs as one tensor_tensor_scan per (d-block, time-chunk);
partition replication of delta/du (64->128) and B/C (16->128) is done with
0/1 matmuls on the tensor engine; dA = exp(A_p * delta) uses the activation
engine's per-partition scale; y = sum_n C*h contracts n back via 0/1
matmuls accumulating into one PSUM tile per 64-channel group.

The 192 inner channels are split as 128 + 64 rows so every partition
offset is quad-aligned (0/64), which the engines require.
"""

import os
import sys
from contextlib import ExitStack

import numpy as np

for _p in ("/opt/trn_rl_repo",):
    if _p not in sys.path and os.path.isdir(_p):
        sys.path.append(_p)

import concourse.bass as bass
import concourse.tile as tile
from concourse import bacc, mybir

F32 = mybir.dt.float32
F32R = mybir.dt.float32r
BF16 = mybir.dt.bfloat16
AL = mybir.AluOpType
AF = mybir.ActivationFunctionType


def _r(ap):
    """View an fp32 AP as float32r: single-pass PE matmul (4x faster for
    moving dim >= 256) at tf32-like precision, plenty for this tolerance."""
    return ap.bitcast(F32R)

# Problem constants
B, H, W, CM = 4, 64, 64, 96
L = H * W  # 4096
D = 192  # d_inner
N = 16  # d_state
RK = 6  # dt_rank
TC = 512  # time-chunk
NCH = L // TC  # 8
NDB = D // 8  # 24 d-blocks
NG = 3  # groups of 64 channels
GDB = NDB // NG  # 8 d-blocks per group
HS = [128, 64]  # d_inner row split
HOF = [0, 128]  # absolute channel offset per half
# group -> (half index, row offset within half)
GMAP = [(0, 0), (0, 64), (1, 0)]
WP = W + 2  # padded row stride for conv


def _rev(ap):
    """Reverse an AP along its last (free) dim."""
    return ap[:, ::-1]


def build_kernel(ctx: ExitStack, tc: "tile.TileContext", io: dict):
    nc = tc.nc

    # ---------------- weight / constant loads ----------------
    wpool = ctx.enter_context(tc.tile_pool(name="wpool", bufs=1))

    w_int = wpool.tile([96, 384], F32R, name="w_int")
    nc.sync.dma_start(w_int[:], io["w_inT"])

    # B/C projections with 16->128 row replication folded in (host-tiled),
    # and the dt projection folded through x_proj (host-matmul'd).
    xpb_t, xpc_t, dtw_t = [], [], []
    for hh in range(2):
        hsl = slice(HOF[hh], HOF[hh] + HS[hh])
        t = wpool.tile([HS[hh], 128], BF16, name=f"xpb_t{hh}")
        nc.sync.dma_start(t[:], io["xpb_wT"][hsl, :])
        xpb_t.append(t)
        t = wpool.tile([HS[hh], 128], BF16, name=f"xpc_t{hh}")
        nc.sync.dma_start(t[:], io["xpc_wT"][hsl, :])
        xpc_t.append(t)
        t = wpool.tile([HS[hh], 192], BF16, name=f"dtw_t{hh}")
        nc.sync.dma_start(t[:], io["dtw_fullT"][hsl, :])
        dtw_t.append(t)

    wout_t = []
    for hh in range(2):
        t = wpool.tile([HS[hh], 96], F32R, name=f"wout_t{hh}")
        nc.sync.dma_start(t[:], io["w_outT"][HOF[hh] : HOF[hh] + HS[hh], :])
        wout_t.append(t)

    def vec_col(name):
        tiles = []
        for hh in range(2):
            t = wpool.tile([HS[hh], 1], F32, name=f"{name}{hh}")
            nc.sync.dma_start(
                t[:],
                io[name][HOF[hh] : HOF[hh] + HS[hh]].rearrange("(p one) -> p one", one=1),
            )
            tiles.append(t)
        return tiles

    dtb = vec_col("dt_proj_b")
    convb = vec_col("conv_b")
    d2 = vec_col("d2")

    a_dn = wpool.tile([128, NDB], F32, name="a_dn")
    nc.sync.dma_start(a_dn[:], io["a_dn"][:])
    # r64 rows are duplicated (0..63 == 64..127) so the lhsT slice can sit
    # at the same base partition as its rhs (a group-base requirement).
    r64 = []  # [j]: [128, 128]; rows k: (k%64 == j*8 + p//16)
    rt64 = []  # [j]: [128, 64] n-contraction lhsT into rows j*8..j*8+8
    for j in range(GDB):
        t = wpool.tile([128, 128], BF16, name=f"r64_{j}")
        nc.sync.dma_start(t[:], io["r64"][j])
        r64.append(t)
        t2 = wpool.tile([128, 64], BF16, name=f"rt64_{j}")
        nc.sync.dma_start(t2[:], io["rt64"][j])
        rt64.append(t2)
    ident = wpool.tile([128, 128], F32R, name="ident")
    nc.sync.dma_start(ident[:], io["ident"][:])

    # ---------------- persistent big buffers ----------------
    ppool = ctx.enter_context(tc.tile_pool(name="persist", bufs=1))
    xT = ppool.tile([96, L], F32, name="xT")  # x transposed (ch, t)
    xc = [ppool.tile([HS[hh], L], BF16, name=f"xc{hh}") for hh in range(2)]
    y_sb = [ppool.tile([HS[hh], L], BF16, name=f"y{hh}") for hh in range(2)]
    b_rep = ppool.tile([128, L], BF16, name="b_rep")
    c_rep = ppool.tile([128, L], BF16, name="c_rep")
    # softplus(dt) for all inner channels, precomputed once (phase 3.5)
    del_sb = [ppool.tile([HS[hh], L], BF16, name=f"del{hh}") for hh in range(2)]

    # ================= phase 1: transpose x + input projection =================
    with (
        tc.tile_pool(name="padpool", bufs=1) as padpool,
        tc.tile_pool(name="cwpool", bufs=1) as cwpool,
    ):
        # conv weights: lhsT [d_in HS[ih], d_out HS[oh]] per (ih, oh, kh, kw)
        cw = {}
        for ih in range(2):
            for oh in range(2):
                for kh in range(3):
                    for kw in range(3):
                        t = cwpool.tile([HS[ih], HS[oh]], F32R, name=f"cw{ih}{oh}{kh}{kw}")
                        src = io["conv_wT"][
                            kh,
                            kw,
                            HOF[ih] : HOF[ih] + HS[ih],
                            HOF[oh] : HOF[oh] + HS[oh],
                        ]
                        nc.sync.dma_start(t[:], src)
                        cw[(ih, oh, kh, kw)] = t

        xp_pad = [
            padpool.tile([HS[hh], (H + 2) * WP], F32, name=f"xp_pad{hh}")
            for hh in range(2)
        ]
        for hh in range(2):
            nc.gpsimd.memset(xp_pad[hh][:], 0.0)

        with (
            tc.tile_pool(name="p1sb", bufs=3) as p1sb,
            tc.tile_pool(name="p1ps", bufs=2, space="PSUM") as p1ps,
        ):
            for m in range(L // 128):
                xt = p1sb.tile([128, 96], F32R, tag="xt")
                nc.sync.dma_start(xt[:], io["x"][m * 128 : (m + 1) * 128, :])
                ps_t = p1ps.tile([96, 128], F32R, tag="ps_t")
                nc.tensor.transpose(ps_t[:], xt[:], ident[:])
                nc.scalar.copy(_r(xT[:, m * 128 : (m + 1) * 128]), ps_t[:].bitcast(F32))

            for ch in range(NCH):
                tsl = slice(ch * TC, (ch + 1) * TC)
                for oh in range(2):
                    ps = p1ps.tile([HS[oh], TC], F32, tag=f"ps_ip{oh}")
                    nc.tensor.matmul(
                        ps[:],
                        _r(w_int[:, HOF[oh] : HOF[oh] + HS[oh]]),
                        _r(xT[:, tsl]),
                        start=True,
                        stop=True,
                    )
                    # write into padded conv buffer rows [ch*8+1..ch*8+8], cols 1..64
                    dst = (
                        xp_pad[oh][:]
                        .rearrange("p (h w) -> p h w", w=WP)[
                            :, ch * 8 + 1 : ch * 8 + 9, 1 : W + 1
                        ]
                    )
                    nc.scalar.copy(_r(dst), ps[:])

        # ================= phase 2: 3x3 conv + bias + silu =================
        with tc.tile_pool(name="p2ps", bufs=2, space="PSUM") as p2ps:
            for ch in range(NCH):
                tsl = slice(ch * TC, (ch + 1) * TC)
                for oh in range(2):
                    ps = p2ps.tile([HS[oh], TC], F32, tag=f"ps_cv{oh}")
                    first = True
                    for ih in range(2):
                        for kh in range(3):
                            for kw in range(3):
                                rhs = (
                                    xp_pad[ih][:]
                                    .rearrange("p (h w) -> p h w", w=WP)[
                                        :, ch * 8 + kh : ch * 8 + kh + 8, kw : kw + W
                                    ]
                                )
                                last = ih == 1 and kh == 2 and kw == 2
                                nc.tensor.matmul(
                                    ps[:],
                                    _r(cw[(ih, oh, kh, kw)][:]),
                                    _r(rhs),
                                    start=first,
                                    stop=last,
                                )
                                first = False
                    nc.scalar.activation(
                        xc[oh][:, tsl], ps[:], AF.Silu, bias=convb[oh][:, 0:1]
                    )

    # ============ phase 3: B/C projection (replication folded in) ============
    with tc.tile_pool(name="p3ps", bufs=2, space="PSUM") as p3ps:
        for ch in range(NCH):
            tsl = slice(ch * TC, (ch + 1) * TC)
            ps_b = p3ps.tile([128, TC], F32, tag="ps_bc")
            nc.tensor.matmul(ps_b[:], xpb_t[0][:], xc[0][:, tsl], start=True, stop=False)
            nc.tensor.matmul(ps_b[:], xpb_t[1][:], xc[1][:, tsl], start=False, stop=True)
            nc.scalar.copy(b_rep[:, tsl], ps_b[:])
            ps_c = p3ps.tile([128, TC], F32, tag="ps_bc")
            nc.tensor.matmul(ps_c[:], xpc_t[0][:], xc[0][:, tsl], start=True, stop=False)
            nc.tensor.matmul(ps_c[:], xpc_t[1][:], xc[1][:, tsl], start=False, stop=True)
            nc.scalar.copy(c_rep[:, tsl], ps_c[:])

        # ---- phase 3.5: delta prologue: del = softplus(dtw @ xc + dtb) ----
        # All Exp activations batched, then two full-length in-place Ln's, so
        # the activation table is swapped twice total instead of per chunk.
        for ch in range(NCH):
            tsl = slice(ch * TC, (ch + 1) * TC)
            for hh in range(2):
                osl = slice(HOF[hh], HOF[hh] + HS[hh])
                ps = p3ps.tile([HS[hh], TC], F32, tag=f"ps35{hh}")
                nc.tensor.matmul(
                    ps[:], dtw_t[0][:, osl], xc[0][:, tsl],
                    start=True, stop=False,
                )
                nc.tensor.matmul(
                    ps[:], dtw_t[1][:, osl], xc[1][:, tsl],
                    start=False, stop=True,
                )
                nc.scalar.activation(
                    del_sb[hh][:, tsl], ps[:], AF.Exp, bias=dtb[hh][:, 0:1]
                )
        for ch in range(NCH):
            tsl = slice(ch * TC, (ch + 1) * TC)
            for hh in range(2):
                nc.scalar.activation(
                    del_sb[hh][:, tsl], del_sb[hh][:, tsl], AF.Ln, bias=1.0
                )

    # ================= phase 4: selective scan (fwd + rev) =================
    # Forward pass computes dA/dBu (bf16), scans, and spills dBu to DRAM;
    # the reverse pass reloads dBu and recomputes dA (matmul on the tensor
    # engine + exp on the idle scalar engine beats 48MB of extra DMA under
    # the activity throttle).
    JTC = GDB * TC  # per-(group, chunk) staging block: all 8 d-blocks
    with (
        tc.tile_pool(name="spill", bufs=1, space="DRAM") as dpool,
        tc.tile_pool(name="scps", bufs=2, space="PSUM") as scps,
        tc.tile_pool(name="scpsy", bufs=2, space="PSUM") as scpsy,
        tc.tile_pool(name="scsb", bufs=2) as scsb,
        tc.tile_pool(name="hpool", bufs=1) as hpool,
    ):
        spB = {}
        for g in range(NG):
            for c in range(NCH):
                spB[(g, c)] = dpool.tile([128, JTC], BF16, name=f"spB{g}_{c}")

        def y_writeback(g, hh, gr0, tsl, psY, rev):
            # f32r/bf16 matmuls cannot target PSUM partition offset 64, so
            # psY always sits at partitions 0:64; group 1's result is shifted
            # to partitions 64:128 with an SBUF-to-SBUF DMA (DMA cannot read
            # PSUM, hence the scalar-copy bounce).
            ysl = y_sb[hh][gr0 : gr0 + 64, tsl]
            if gr0 == 64:
                ytmp0 = scsb.tile([64, TC], BF16, tag="ytmp0")
                nc.scalar.copy(ytmp0[:], psY[:])
                if rev:
                    ytmp = scsb.tile([128, TC], BF16, tag="ytmp")
                    nc.sync.dma_start(ytmp[64:128, :], ytmp0[:])
                    nc.vector.tensor_tensor(ysl, ysl, _rev(ytmp[64:128, :]), AL.add)
                else:
                    nc.sync.dma_start(ysl, ytmp0[:])
            elif rev:
                nc.vector.tensor_tensor(ysl, ysl, _rev(psY[:]), AL.add)
            else:
                nc.scalar.copy(ysl, psY[:])

        # ---- forward + reverse interleaved per group: the reverse pass of
        # group g (DVE/pool heavy) overlaps the forward prep of group g+1
        # (tensor/scalar heavy) instead of waiting for all forward groups.
        with (
            tc.tile_pool(name="stpool", bufs=2) as stpool,
            tc.tile_pool(name="ldpool", bufs=2) as ldpool,
        ):
            for g in range(NG):
                hh, gr0 = GMAP[g]
                h_prev = {}
                gp = slice(gr0, gr0 + 64)
                dBu_keep = {}
                for it in range(NCH):
                    tsl = slice(it * TC, (it + 1) * TC)
                    du_c = scsb.tile([128, TC], BF16, tag="du_c")
                    nc.gpsimd.tensor_tensor(
                        du_c[gp, :], del_sb[hh][gp, tsl], xc[hh][gp, tsl], AL.mult
                    )
                    dA_st = stpool.tile([128, JTC], BF16, tag="dA_st")
                    dBu_st = stpool.tile([128, JTC], BF16, tag="dBu_st")
                    psY = scpsy.tile([64, TC], F32, tag="psY")

                    for j in range(GDB):
                        db = g * GDB + j
                        jsl = slice(j * TC, (j + 1) * TC)

                        ps_d = scps.tile([128, TC], F32, tag="ps_d")
                        nc.tensor.matmul(
                            ps_d[:], r64[j][gp, :], del_sb[hh][gp, tsl],
                            start=True, stop=True,
                        )
                        nc.scalar.activation(
                            dA_st[:, jsl], ps_d[:], AF.Exp, scale=a_dn[:, db : db + 1]
                        )

                        ps_u = scps.tile([128, TC], F32, tag="ps_u")
                        nc.tensor.matmul(
                            ps_u[:], r64[j][gp, :], du_c[gp, :], start=True, stop=True
                        )
                        nc.vector.scalar_tensor_tensor(
                            dBu_st[:, jsl], ps_u[:], 1.0, b_rep[:, tsl],
                            AL.mult, AL.mult,
                        )

                        h = hpool.tile([128, TC], BF16, tag=f"h{j}", bufs=2)
                        init = 0.0 if it == 0 else h_prev[j][:, TC - 1 : TC]
                        nc.vector.tensor_tensor_scan(
                            h[:], dA_st[:, jsl], dBu_st[:, jsl], init, AL.mult, AL.add
                        )
                        h_prev[j] = h

                        o = scsb.tile([128, TC], BF16, tag="o")
                        nc.gpsimd.tensor_tensor(o[:], h[:], c_rep[:, tsl], AL.mult)

                        nc.tensor.matmul(
                            psY[:], rt64[j][:], o[:],
                            start=(j == 0), stop=(j == GDB - 1),
                        )

                    if it >= NCH - 2:
                        # stpool is double-buffered: the last two units' dBu
                        # staging tiles are both still resident when the
                        # reverse pass starts; read them directly.
                        dBu_keep[it] = dBu_st
                    else:
                        nc.sync.dma_start(spB[(g, it)][:], dBu_st[:])
                    y_writeback(g, hh, gr0, tsl, psY, rev=False)

                # ---- reverse (reload dBu, recompute dA) ----
                h_prev = {}
                for it in range(NCH):
                    c = NCH - 1 - it
                    tsl = slice(c * TC, (c + 1) * TC)
                    if c in dBu_keep:
                        dBu_ld = dBu_keep[c]
                    else:
                        dBu_ld = ldpool.tile([128, JTC], BF16, tag="dBu_ld", bufs=3)
                        nc.sync.dma_start(dBu_ld[:], spB[(g, c)][:])
                    psY = scpsy.tile([64, TC], F32, tag="psY")

                    for j in range(GDB):
                        db = g * GDB + j
                        jsl = slice(j * TC, (j + 1) * TC)
                        ps_d = scps.tile([128, TC], F32, tag="ps_d")
                        nc.tensor.matmul(
                            ps_d[:], r64[j][gp, :], del_sb[hh][gp, tsl],
                            start=True, stop=True,
                        )
                        dA_t = scsb.tile([128, TC], BF16, tag="dA_t")
                        nc.scalar.activation(
                            dA_t[:], ps_d[:], AF.Exp, scale=a_dn[:, db : db + 1]
                        )
                        h = hpool.tile([128, TC], BF16, tag=f"h{j}", bufs=2)
                        init = 0.0 if it == 0 else h_prev[j][:, TC - 1 : TC]
                        nc.vector.tensor_tensor_scan(
                            h[:], _rev(dA_t[:]), _rev(dBu_ld[:, jsl]),
                            init, AL.mult, AL.add,
                        )
                        h_prev[j] = h

                        o = scsb.tile([128, TC], BF16, tag="o")
                        nc.gpsimd.tensor_tensor(o[:], h[:], _rev(c_rep[:, tsl]), AL.mult)

                        nc.tensor.matmul(
                            psY[:], rt64[j][:], o[:],
                            start=(j == 0), stop=(j == GDB - 1),
                        )

                    y_writeback(g, hh, gr0, tsl, psY, rev=True)

    # ================= phase 5: D*u, gate with silu(z), out-proj =================
    # Out-projection is computed transposed ([96, L], moving dim 512 so the
    # f32r single-pass matmul applies); the host transposes when assembling.
    out_sb = ppool.tile([96, L], F32, name="out_sb")
    with (
        tc.tile_pool(name="p6ps", bufs=2, space="PSUM") as p6ps,
        tc.tile_pool(name="p6sb", bufs=3) as p6sb,
    ):
        for ch in range(NCH):
            tsl = slice(ch * TC, (ch + 1) * TC)
            yg = []
            for hh in range(2):
                ps_z = p6ps.tile([HS[hh], TC], F32, tag=f"ps_z{hh}")
                nc.tensor.matmul(
                    ps_z[:],
                    _r(w_int[:, 192 + HOF[hh] : 192 + HOF[hh] + HS[hh]]),
                    _r(xT[:, tsl]),
                    start=True,
                    stop=True,
                )
                z_act = p6sb.tile([HS[hh], TC], F32, tag=f"z_act{hh}")
                nc.scalar.activation(z_act[:], ps_z[:], AF.Silu)

                yf = p6sb.tile([HS[hh], TC], F32, tag=f"yf{hh}")
                nc.vector.scalar_tensor_tensor(
                    yf[:], xc[hh][:, tsl], d2[hh][:, 0:1], y_sb[hh][:, tsl],
                    AL.mult, AL.add,
                )
                g = p6sb.tile([HS[hh], TC], F32, tag=f"yg{hh}")
                nc.gpsimd.tensor_tensor(_r(g[:]), yf[:], z_act[:], AL.mult)
                yg.append(g)

            ps_o = p6ps.tile([96, TC], F32, tag="ps_o")
            nc.tensor.matmul(ps_o[:], _r(wout_t[0][:]), _r(yg[0][:]), start=True, stop=False)
            nc.tensor.matmul(ps_o[:], _r(wout_t[1][:]), _r(yg[1][:]), start=False, stop=True)
            nc.vector.tensor_copy(out_sb[:, tsl], ps_o[:])
        nc.sync.dma_start(io["out"][:], out_sb[:])


# ---------------------------------------------------------------------------
# host-side wrapper
# ---------------------------------------------------------------------------

def _host_constants(A_logs):
    A = -np.exp(np.asarray(A_logs, np.float32))  # (192, 16)
    p = np.arange(128)
    a_dn = np.zeros((128, NDB), np.float32)
    for db in range(NDB):
        a_dn[:, db] = A[db * 8 + p // 16, p % 16]
    r64 = np.zeros((GDB, 128, 128), np.float32)
    rt64 = np.zeros((GDB, 128, 64), np.float32)
    for j in range(GDB):
        r64[j] = (np.arange(128) % 64)[:, None] == (j * 8 + p // 16)[None, :]
        rt64[j] = (j * 8 + p // 16)[:, None] == np.arange(64)[None, :]
    ident = np.eye(128, dtype=np.float32)
    return a_dn, r64, rt64, ident


_NC_CACHE = {}


def _get_nc():
    if "nc" in _NC_CACHE:
        return _NC_CACHE["nc"]
    nc = bacc.Bacc(
        "TRN2", target_bir_lowering=False, debug=False, enable_asserts=False,
        num_devices=8,
    )
    io = {
        "x": nc.dram_tensor("x", [L, CM], F32R, kind="ExternalInput").ap(),
        "w_inT": nc.dram_tensor("w_inT", [CM, 2 * D], F32R, kind="ExternalInput").ap(),
        "conv_wT": nc.dram_tensor("conv_wT", [3, 3, D, D], F32R, kind="ExternalInput").ap(),
        "conv_b": nc.dram_tensor("conv_b", [D], F32, kind="ExternalInput").ap(),
        "xpb_wT": nc.dram_tensor("xpb_wT", [D, 128], BF16, kind="ExternalInput").ap(),
        "xpc_wT": nc.dram_tensor("xpc_wT", [D, 128], BF16, kind="ExternalInput").ap(),
        "dtw_fullT": nc.dram_tensor("dtw_fullT", [D, D], BF16, kind="ExternalInput").ap(),
        "dt_proj_b": nc.dram_tensor("dt_proj_b", [D], F32, kind="ExternalInput").ap(),
        "d2": nc.dram_tensor("d2", [D], F32, kind="ExternalInput").ap(),
        "w_outT": nc.dram_tensor("w_outT", [D, CM], F32R, kind="ExternalInput").ap(),
        "a_dn": nc.dram_tensor("a_dn", [128, NDB], F32, kind="ExternalInput").ap(),
        "r64": nc.dram_tensor("r64", [GDB, 128, 128], BF16, kind="ExternalInput").ap(),
        "rt64": nc.dram_tensor("rt64", [GDB, 128, 64], BF16, kind="ExternalInput").ap(),
        "ident": nc.dram_tensor("ident", [128, 128], F32R, kind="ExternalInput").ap(),
        "out": nc.dram_tensor("out", [CM, L], F32, kind="ExternalOutput").ap(),
    }
    with tile.TileContext(nc) as tc:
        with ExitStack() as ctx:
            build_kernel(ctx, tc, io)
    nc.compile()
    _NC_CACHE["nc"] = nc
    _NC_CACHE["io_names"] = list(io.keys())
    return nc


def make_in_maps(x, W_in, conv_w, conv_b, x_proj_w, dt_proj_w, dt_proj_b, A_logs,
                 Ds, W_out):
    import ml_dtypes

    f = lambda a: np.ascontiguousarray(np.asarray(a, dtype=np.float32))
    a_dn, r64, rt64, ident = _host_constants(A_logs)
    xpw = f(x_proj_w)
    common = {
        "w_inT": np.ascontiguousarray(f(W_in).T), "conv_b": f(conv_b),
        "xpb_wT": np.ascontiguousarray(np.tile(xpw[RK : RK + N], (8, 1)).T.astype(ml_dtypes.bfloat16)),
        "xpc_wT": np.ascontiguousarray(np.tile(xpw[RK + N : RK + 2 * N], (8, 1)).T.astype(ml_dtypes.bfloat16)),
        "dtw_fullT": np.ascontiguousarray((f(dt_proj_w) @ xpw[:RK]).T.astype(ml_dtypes.bfloat16)),
        "dt_proj_b": f(dt_proj_b),
        "d2": f(Ds) * 2.0, "w_outT": np.ascontiguousarray(f(W_out).T), "a_dn": a_dn,
        "r64": np.ascontiguousarray(r64.astype(ml_dtypes.bfloat16)),
        "rt64": np.ascontiguousarray(rt64.astype(ml_dtypes.bfloat16)),
        "ident": ident,
    }
    x = f(x)
    cw = f(conv_w)
    cw_t = np.ascontiguousarray(cw.transpose(0, 1, 3, 2))
    in_maps = []
    for c in range(8):
        b, lay = c // 2, c % 2
        xv = x[b] if lay == 0 else np.ascontiguousarray(x[b].transpose(1, 0, 2))
        cwl = cw if lay == 0 else cw_t
        in_maps.append(
            {**common, "x": xv.reshape(L, CM),
             "conv_wT": np.ascontiguousarray(cwl.transpose(2, 3, 1, 0))}
        )
    return in_maps


def assemble(parts):
    out = np.zeros((B, L, CM), np.float32)
    for c in range(8):
        b, lay = c // 2, c % 2
        p = parts[c].T  # device emits [96, L]
        if lay:
            p = p.reshape(W, H, CM).transpose(1, 0, 2).reshape(L, CM)
        out[b] += p
    return out.reshape(B, H, W, CM)


def kernel(**inputs):
    from concourse.bass_utils import run_bass_kernel_spmd

    nc = _get_nc()
    in_maps = make_in_maps(**inputs)
    res = run_bass_kernel_spmd(nc, in_maps, list(range(8)))
    return assemble([res.results[c]["out"] for c in range(8)])



# revision 14
# speedup vs baseline: 1.5223x; 1.5223x over previous
"""BiMamba2D (VMamba-style 4-direction selective scan) Trainium2 Bass kernel.

Sharding: 8 cores = 4 batches x 2 scan layouts (hw / wh); each core runs both
time directions of its layout and emits a partial (96, L) output; the host
sums partials.

Scan-phase design (v2):
  * State layout is n-MAJOR: partition p of a d-block holds (state n = p//8,
    channel c = p%8).  This makes the 8->128 partition replication of
    delta / delta*u a chain of 5 partition-contiguous SBUF->SBUF DMAs
    (doubling), entirely off the compute engines.
  * All elementwise work lives on the DVE in bf16 2x mode; GpSimd is idle
    (measured: gpsimd ops and DVE scans mutually block on the shared SBUF
    port pair, nearly serializing the two engines).
  * Scans are single full-L [128, 4096] tensor_tensor_scan ops (48 total):
    ~12% cheaper per element than chunked scans, no h chaining, and dA/dBu
    are computed once and read by both the forward scan and the reversed-AP
    backward scan (no recompute, no DRAM spill).
  * y = sum_n C*h accumulates via 0/1 matmuls into 8 PSUM banks (one per
    time chunk); both directions of all 8 d-blocks of a group accumulate
    into the same banks, so writeback is one PSUM->SBUF copy per chunk.
"""

import os
import sys
from contextlib import ExitStack

import numpy as np

for _p in ("/opt/trn_rl_repo",):
    if _p not in sys.path and os.path.isdir(_p):
        sys.path.append(_p)

import concourse.bass as bass
import concourse.tile as tile
from concourse import bacc, mybir

F32 = mybir.dt.float32
F32R = mybir.dt.float32r
BF16 = mybir.dt.bfloat16
AL = mybir.AluOpType
AF = mybir.ActivationFunctionType

DEBUG = os.environ.get("KDBG", "0") not in ("0", "5")
DEBUG_KEEP = os.environ.get("KDBG") == "5"
DEBUG_J = os.environ.get("KDBG") in ("1", "3")   # per-j dumps
DEBUG_P4 = os.environ.get("KDBG") in ("1", "2")  # end-of-phase-4 dumps
DBG_G = int(os.environ.get("KDBG_G", "0"))
DBG_J_IDX = int(os.environ.get("KDBG_JIDX", "0"))


def _r(ap):
    """View an fp32 AP as float32r: single-pass PE matmul at tf32-like
    precision, plenty for this tolerance."""
    return ap.bitcast(F32R)

# Problem constants
B, H, W, CM = 4, 64, 64, 96
L = H * W  # 4096
D = 192  # d_inner
N = 16  # d_state
RK = 6  # dt_rank
TC = 512  # time-chunk (PSUM bank size)
NCH = L // TC  # 8
NG = 3  # groups of 64 channels
GDB = 8  # d-blocks per group
HS = [128, 64]  # d_inner row split
HOF = [0, 128]  # absolute channel offset per half
# group -> (half index, row offset within half)
GMAP = [(0, 0), (0, 64), (1, 0)]
WP = W + 2  # padded row stride for conv


def build_kernel(ctx: ExitStack, tc: "tile.TileContext", io: dict):
    nc = tc.nc

    # ---------------- weight / constant loads ----------------
    wpool = ctx.enter_context(tc.tile_pool(name="wpool", bufs=1))

    w_int = wpool.tile([96, 384], F32R, name="w_int")
    nc.sync.dma_start(w_int[:], io["w_inT"])

    # B/C projections with 16->128 n-major row replication folded in
    # (host-tiled), and the dt projection folded through x_proj.
    xpb_t, xpc_t, dtw_t = [], [], []
    for hh in range(2):
        hsl = slice(HOF[hh], HOF[hh] + HS[hh])
        t = wpool.tile([HS[hh], 128], BF16, name=f"xpb_t{hh}")
        nc.sync.dma_start(t[:], io["xpb_wT"][hsl, :])
        xpb_t.append(t)
        t = wpool.tile([HS[hh], 128], BF16, name=f"xpc_t{hh}")
        nc.sync.dma_start(t[:], io["xpc_wT"][hsl, :])
        xpc_t.append(t)
        t = wpool.tile([HS[hh], 192], BF16, name=f"dtw_t{hh}")
        nc.sync.dma_start(t[:], io["dtw_fullT"][hsl, :])
        dtw_t.append(t)

    wout_t = []
    for hh in range(2):
        t = wpool.tile([HS[hh], 96], F32R, name=f"wout_t{hh}")
        nc.sync.dma_start(t[:], io["w_outT"][HOF[hh] : HOF[hh] + HS[hh], :])
        wout_t.append(t)

    def vec_col(name):
        tiles = []
        for hh in range(2):
            t = wpool.tile([HS[hh], 1], F32, name=f"{name}{hh}")
            nc.sync.dma_start(
                t[:],
                io[name][HOF[hh] : HOF[hh] + HS[hh]].rearrange("(p one) -> p one", one=1),
            )
            tiles.append(t)
        return tiles

    dtb = vec_col("dt_proj_b")
    convb = vec_col("conv_b")
    d2 = vec_col("d2")

    a_col = wpool.tile([128, 1], F32, name="a_col")
    nc.sync.dma_start(a_col[:], io["a_col"][:])
    rt64 = []  # [j]: [128, 64] n-contraction lhsT: 1 iff d64 == j*8 + p%8
    for j in range(GDB):
        t = wpool.tile([128, 64], BF16, name=f"rt64_{j}")
        nc.sync.dma_start(t[:], io["rt64"][j])
        rt64.append(t)
    ident = wpool.tile([128, 128], F32R, name="ident")
    nc.sync.dma_start(ident[:], io["ident"][:])

    # ---------------- persistent big buffers ----------------
    ppool = ctx.enter_context(tc.tile_pool(name="persist", bufs=1))
    xc = [ppool.tile([HS[hh], L], BF16, name=f"xc{hh}") for hh in range(2)]
    y_sb = [ppool.tile([HS[hh], L], BF16, name=f"y{hh}") for hh in range(2)]
    b_rep = ppool.tile([128, L], BF16, name="b_rep")
    c_rep = ppool.tile([128, L], BF16, name="c_rep")
    # softplus(dt) for all inner channels, precomputed once (phase 3.5)
    del_sb = [ppool.tile([HS[hh], L], BF16, name=f"del{hh}") for hh in range(2)]

    # ================= phase 1: transpose x + input projection =================
    with (
        tc.tile_pool(name="p1big", bufs=1) as p1big,
        tc.tile_pool(name="cwpool", bufs=1) as cwpool,
    ):
        xT = p1big.tile([96, L], F32, name="xT")  # x transposed (scoped to P1/P2)
        # conv weights: lhsT [d_in HS[ih], d_out HS[oh]] per (ih, oh, kh, kw)
        cw = {}
        for ih in range(2):
            for oh in range(2):
                for kh in range(3):
                    for kw in range(3):
                        t = cwpool.tile([HS[ih], HS[oh]], F32R, name=f"cw{ih}{oh}{kh}{kw}")
                        src = io["conv_wT"][
                            kh,
                            kw,
                            HOF[ih] : HOF[ih] + HS[ih],
                            HOF[oh] : HOF[oh] + HS[oh],
                        ]
                        nc.sync.dma_start(t[:], src)
                        cw[(ih, oh, kh, kw)] = t

        xp_pad = [
            p1big.tile([HS[hh], (H + 2) * WP], F32, name=f"xp_pad{hh}")
            for hh in range(2)
        ]
        for hh in range(2):
            nc.gpsimd.memset(xp_pad[hh][:], 0.0)

        with (
            tc.tile_pool(name="p1sb", bufs=3) as p1sb,
            tc.tile_pool(name="p1ps", bufs=2, space="PSUM") as p1ps,
        ):
            for m in range(L // 128):
                xt = p1sb.tile([128, 96], F32R, tag="xt")
                nc.sync.dma_start(xt[:], io["x"][m * 128 : (m + 1) * 128, :])
                ps_t = p1ps.tile([96, 128], F32R, tag="ps_t")
                nc.tensor.transpose(ps_t[:], xt[:], ident[:])
                nc.scalar.copy(_r(xT[:, m * 128 : (m + 1) * 128]), ps_t[:].bitcast(F32))

            for ch in range(NCH):
                tsl = slice(ch * TC, (ch + 1) * TC)
                for oh in range(2):
                    ps = p1ps.tile([HS[oh], TC], F32, tag=f"ps_ip{oh}")
                    nc.tensor.matmul(
                        ps[:],
                        _r(w_int[:, HOF[oh] : HOF[oh] + HS[oh]]),
                        _r(xT[:, tsl]),
                        start=True,
                        stop=True,
                    )
                    # write into padded conv buffer rows [ch*8+1..ch*8+8], cols 1..64
                    dst = (
                        xp_pad[oh][:]
                        .rearrange("p (h w) -> p h w", w=WP)[
                            :, ch * 8 + 1 : ch * 8 + 9, 1 : W + 1
                        ]
                    )
                    nc.scalar.copy(_r(dst), ps[:])

        # ================= phase 2: 3x3 conv + bias + silu =================
        with tc.tile_pool(name="p2ps", bufs=2, space="PSUM") as p2ps:
            for ch in range(NCH):
                tsl = slice(ch * TC, (ch + 1) * TC)
                for oh in range(2):
                    ps = p2ps.tile([HS[oh], TC], F32, tag=f"ps_cv{oh}")
                    first = True
                    for ih in range(2):
                        for kh in range(3):
                            for kw in range(3):
                                rhs = (
                                    xp_pad[ih][:]
                                    .rearrange("p (h w) -> p h w", w=WP)[
                                        :, ch * 8 + kh : ch * 8 + kh + 8, kw : kw + W
                                    ]
                                )
                                last = ih == 1 and kh == 2 and kw == 2
                                nc.tensor.matmul(
                                    ps[:],
                                    _r(cw[(ih, oh, kh, kw)][:]),
                                    _r(rhs),
                                    start=first,
                                    stop=last,
                                )
                                first = False
                    nc.scalar.activation(
                        xc[oh][:, tsl], ps[:], AF.Silu, bias=convb[oh][:, 0:1]
                    )

    # ============ phase 3: B/C projection (replication folded in) ============
    with tc.tile_pool(name="p3ps", bufs=2, space="PSUM") as p3ps:
        for ch in range(NCH):
            tsl = slice(ch * TC, (ch + 1) * TC)
            ps_b = p3ps.tile([128, TC], F32, tag="ps_bc")
            nc.tensor.matmul(ps_b[:], xpb_t[0][:], xc[0][:, tsl], start=True, stop=False)
            nc.tensor.matmul(ps_b[:], xpb_t[1][:], xc[1][:, tsl], start=False, stop=True)
            nc.scalar.copy(b_rep[:, tsl], ps_b[:])
            ps_c = p3ps.tile([128, TC], F32, tag="ps_bc")
            nc.tensor.matmul(ps_c[:], xpc_t[0][:], xc[0][:, tsl], start=True, stop=False)
            nc.tensor.matmul(ps_c[:], xpc_t[1][:], xc[1][:, tsl], start=False, stop=True)
            nc.scalar.copy(c_rep[:, tsl], ps_c[:])

        # ---- phase 3.5: delta prologue: del = softplus(dtw @ xc + dtb) ----
        for ch in range(NCH):
            tsl = slice(ch * TC, (ch + 1) * TC)
            for hh in range(2):
                osl = slice(HOF[hh], HOF[hh] + HS[hh])
                ps = p3ps.tile([HS[hh], TC], F32, tag=f"ps35{hh}")
                nc.tensor.matmul(
                    ps[:], dtw_t[0][:, osl], xc[0][:, tsl],
                    start=True, stop=False,
                )
                nc.tensor.matmul(
                    ps[:], dtw_t[1][:, osl], xc[1][:, tsl],
                    start=False, stop=True,
                )
                nc.scalar.activation(
                    del_sb[hh][:, tsl], ps[:], AF.Exp, bias=dtb[hh][:, 0:1]
                )
        for ch in range(NCH):
            tsl = slice(ch * TC, (ch + 1) * TC)
            for hh in range(2):
                nc.scalar.activation(
                    del_sb[hh][:, tsl], del_sb[hh][:, tsl], AF.Ln, bias=1.0
                )

    # ================= phase 4: selective scan (fwd + rev) =================
    # 8 -> 128 partition replication (n-major): spill the group rows to DRAM
    # once, then one DMA per d-block reads them back through a broadcast AP.
    # (Chained same-tile SBUF->SBUF DMAs race on hardware; DRAM round-trip
    # DMA->DMA dependencies are reliable.)
    with (
        tc.tile_pool(name="spillp", bufs=2, space="DRAM") as spillp,
        tc.tile_pool(name="dreppool", bufs=2) as dreppool,
        tc.tile_pool(name="dapool", bufs=2) as dapool,
        tc.tile_pool(name="ureppool", bufs=2) as ureppool,
        tc.tile_pool(name="dbupool", bufs=2) as dbupool,
        tc.tile_pool(name="hpool", bufs=3) as hpool,
        tc.tile_pool(name="opool", bufs=3) as opool,
        tc.tile_pool(name="ducpool", bufs=1) as ducpool,
        tc.tile_pool(name="wbpool", bufs=2) as wbpool,
        tc.tile_pool(name="scpsy", bufs=1, space="PSUM") as scpsy,
    ):
        for g in range(NG):
            hh, gr0 = GMAP[g]
            gp = slice(gr0, gr0 + 64)
            # du = delta * conv-act for the group's 64 channels (full L)
            du_c = ducpool.tile([128, L], BF16, tag="du_c")
            nc.vector.tensor_tensor(
                du_c[gp, :], del_sb[hh][gp, :], xc[hh][gp, :], AL.mult
            )
            del_sp = spillp.tile([64, L], BF16, tag="del_sp")
            nc.sync.dma_start(del_sp[:], del_sb[hh][gp, :])
            du_sp = spillp.tile([64, L], BF16, tag="du_sp")
            nc.sync.dma_start(du_sp[:], du_c[gp, :])
            psY = [
                scpsy.tile([128, TC], F32, tag=f"psY{c}", name=f"psY{c}")
                for c in range(NCH)
            ]

            for j in range(GDB):
                rsl = slice(gr0 + j * 8, gr0 + j * 8 + 8)
                jsl = slice(j * 8, (j + 1) * 8)
                drep = dreppool.tile([128, L], BF16, tag="drep")
                nc.sync.dma_start(
                    drep[:], del_sp[jsl, :].unsqueeze(0).broadcast_to([16, 8, L])
                )
                dA = dapool.tile([128, L], BF16, tag="dA")
                nc.scalar.activation(dA[:], drep[:], AF.Exp, scale=a_col[:, 0:1])

                urep = ureppool.tile([128, L], BF16, tag="urep")
                nc.sync.dma_start(
                    urep[:], du_sp[jsl, :].unsqueeze(0).broadcast_to([16, 8, L])
                )
                dBu = dbupool.tile([128, L], BF16, tag="dBu")
                nc.vector.tensor_tensor(dBu[:], urep[:], b_rep[:], AL.mult)

                if os.environ.get("KDBG") == "5" and g == DBG_G and j == DBG_J_IDX:
                    kp1 = wpool.tile([128, 512], BF16, name="kp1")
                    kp2 = wpool.tile([128, 512], BF16, name="kp2")
                    nc.vector.tensor_copy(kp1[:], drep[:, 0:512])
                    nc.vector.tensor_copy(kp2[:], urep[:, 0:512])
                    nc.sync.dma_start(io["dbg_drep"][:, 0:512], kp1[:])
                    nc.sync.dma_start(io["dbg_urep"][:, 0:512], kp2[:])
                h_f = hpool.tile([128, L], BF16, tag="h")
                nc.vector.tensor_tensor_scan(h_f[:], dA[:], dBu[:], 0.0, AL.mult, AL.add)
                o_f = opool.tile([128, L], BF16, tag="o")
                nc.vector.tensor_tensor(o_f[:], h_f[:], c_rep[:], AL.mult)
                for c in range(NCH):
                    csl = slice(c * TC, (c + 1) * TC)
                    nc.tensor.matmul(
                        psY[c][0:64, :], rt64[j][:], o_f[:, csl],
                        start=(j == 0), stop=False,
                    )

                h_r = hpool.tile([128, L], BF16, tag="h")
                nc.vector.tensor_tensor_scan(
                    h_r[:], dA[:, ::-1], dBu[:, ::-1], 0.0, AL.mult, AL.add
                )
                # time-corrected: o_r[t] = h_r[L-1-t] * C[t]
                o_r = opool.tile([128, L], BF16, tag="o")
                nc.vector.tensor_tensor(o_r[:], h_r[:, ::-1], c_rep[:], AL.mult)
                if DEBUG_J and g == DBG_G and j == DBG_J_IDX:
                    for nm, t in [("dbg_drep", drep), ("dbg_dA", dA),
                                  ("dbg_urep", urep), ("dbg_dBu", dBu),
                                  ("dbg_hf", h_f), ("dbg_of", o_f),
                                  ("dbg_hr", h_r), ("dbg_or", o_r)]:
                        nc.sync.dma_start(io[nm][:], t[:])
                for c in range(NCH):
                    csl = slice(c * TC, (c + 1) * TC)
                    nc.tensor.matmul(
                        psY[c][0:64, :], rt64[j][:], o_r[:, csl],
                        start=False, stop=(j == GDB - 1),
                    )

            # ---- writeback: one PSUM->SBUF copy per chunk ----
            for c in range(NCH):
                csl = slice(c * TC, (c + 1) * TC)
                if gr0 == 0:
                    nc.scalar.copy(y_sb[hh][0:64, csl], psY[c][0:64, :])
                else:
                    # engines cannot shift partitions; bounce via SBUF + DMA
                    wt = wbpool.tile([128, TC], BF16, tag="wt")
                    nc.scalar.copy(wt[0:64, :], psY[c][0:64, :])
                    nc.sync.dma_start(y_sb[hh][64:128, csl], wt[0:64, :])

    # phase-5's scoped pools reuse the scan-phase SBUF/PSUM addresses; fence
    # so nothing in phase 5 can clobber tiles still being read.
    tc.strict_bb_all_engine_barrier()

    if DEBUG_P4:
        nc.sync.dma_start(io["dbg_ysb0"][:], y_sb[0][:])
        nc.sync.dma_start(io["dbg_brep"][:], b_rep[:])
        nc.sync.dma_start(io["dbg_crep"][:], c_rep[:])
        nc.sync.dma_start(io["dbg_del0"][:], del_sb[0][:])
        nc.sync.dma_start(io["dbg_xc0"][:], xc[0][:])

    # ======== phase 5: z-gate, D*u, out-projection (per chunk, DMA out) ========
    with (
        tc.tile_pool(name="p6ps", bufs=2, space="PSUM") as p6ps,
        tc.tile_pool(name="p6sb", bufs=3) as p6sb,
        tc.tile_pool(name="p6xt", bufs=2) as p6xt,
    ):
        for ch in range(NCH):
            tsl = slice(ch * TC, (ch + 1) * TC)
            # re-derive xT chunk (x transposed) for the z projection
            xTc = p6xt.tile([96, TC], F32, tag="xTc")
            for m in range(TC // 128):
                xt = p6sb.tile([128, 96], F32R, tag="xt")
                nc.sync.dma_start(
                    xt[:], io["x"][ch * TC + m * 128 : ch * TC + (m + 1) * 128, :]
                )
                ps_t = p6ps.tile([96, 128], F32R, tag="ps_t")
                nc.tensor.transpose(ps_t[:], xt[:], ident[:])
                nc.scalar.copy(_r(xTc[:, m * 128 : (m + 1) * 128]), ps_t[:].bitcast(F32))

            yg = []
            for hh in range(2):
                ps_z = p6ps.tile([HS[hh], TC], F32, tag=f"ps_z{hh}")
                nc.tensor.matmul(
                    ps_z[:],
                    _r(w_int[:, 192 + HOF[hh] : 192 + HOF[hh] + HS[hh]]),
                    _r(xTc[:]),
                    start=True,
                    stop=True,
                )
                z_act = p6sb.tile([HS[hh], TC], F32, tag=f"z_act{hh}")
                nc.scalar.activation(z_act[:], ps_z[:], AF.Silu)

                yf = p6sb.tile([HS[hh], TC], F32, tag=f"yf{hh}")
                nc.vector.scalar_tensor_tensor(
                    yf[:], xc[hh][:, tsl], d2[hh][:, 0:1], y_sb[hh][:, tsl],
                    AL.mult, AL.add,
                )
                g = p6sb.tile([HS[hh], TC], F32, tag=f"yg{hh}")
                nc.vector.tensor_tensor(_r(g[:]), yf[:], z_act[:], AL.mult)
                yg.append(g)

            ps_o = p6ps.tile([96, TC], F32, tag="ps_o")
            nc.tensor.matmul(ps_o[:], _r(wout_t[0][:]), _r(yg[0][:]), start=True, stop=False)
            nc.tensor.matmul(ps_o[:], _r(wout_t[1][:]), _r(yg[1][:]), start=False, stop=True)
            out_c = p6sb.tile([96, TC], F32, tag="out_c")
            nc.scalar.copy(out_c[:], ps_o[:])
            nc.sync.dma_start(io["out"][:, tsl], out_c[:])


# ---------------------------------------------------------------------------
# host-side wrapper
# ---------------------------------------------------------------------------

def _host_constants(A_logs):
    A = -np.exp(np.asarray(A_logs, np.float32))  # (192, 16)
    p = np.arange(128)
    a_col = A[p % 8, p // 8].reshape(128, 1).astype(np.float32)
    rt64 = np.zeros((GDB, 128, 64), np.float32)
    for j in range(GDB):
        rt64[j] = (j * 8 + p % 8)[:, None] == np.arange(64)[None, :]
    ident = np.eye(128, dtype=np.float32)
    return a_col, rt64, ident


_NC_CACHE = {}


def _get_nc():
    if "nc" in _NC_CACHE:
        return _NC_CACHE["nc"]
    nc = bacc.Bacc(
        "TRN2", target_bir_lowering=False, debug=False, enable_asserts=False,
        num_devices=8,
    )
    io = {
        "x": nc.dram_tensor("x", [L, CM], F32R, kind="ExternalInput").ap(),
        "w_inT": nc.dram_tensor("w_inT", [CM, 2 * D], F32R, kind="ExternalInput").ap(),
        "conv_wT": nc.dram_tensor("conv_wT", [3, 3, D, D], F32R, kind="ExternalInput").ap(),
        "conv_b": nc.dram_tensor("conv_b", [D], F32, kind="ExternalInput").ap(),
        "xpb_wT": nc.dram_tensor("xpb_wT", [D, 128], BF16, kind="ExternalInput").ap(),
        "xpc_wT": nc.dram_tensor("xpc_wT", [D, 128], BF16, kind="ExternalInput").ap(),
        "dtw_fullT": nc.dram_tensor("dtw_fullT", [D, D], BF16, kind="ExternalInput").ap(),
        "dt_proj_b": nc.dram_tensor("dt_proj_b", [D], F32, kind="ExternalInput").ap(),
        "d2": nc.dram_tensor("d2", [D], F32, kind="ExternalInput").ap(),
        "w_outT": nc.dram_tensor("w_outT", [D, CM], F32R, kind="ExternalInput").ap(),
        "a_col": nc.dram_tensor("a_col", [128, 1], F32, kind="ExternalInput").ap(),
        "rt64": nc.dram_tensor("rt64", [GDB, 128, 64], BF16, kind="ExternalInput").ap(),
        "ident": nc.dram_tensor("ident", [128, 128], F32R, kind="ExternalInput").ap(),
        "out": nc.dram_tensor("out", [CM, L], F32, kind="ExternalOutput").ap(),
    }
    if DEBUG or DEBUG_KEEP:
        for nm in ("dbg_drep", "dbg_dA", "dbg_urep", "dbg_dBu", "dbg_hf",
                   "dbg_of", "dbg_hr", "dbg_or"):
            io[nm] = nc.dram_tensor(nm, [128, L], BF16, kind="ExternalOutput").ap()
        io["dbg_ysb0"] = nc.dram_tensor("dbg_ysb0", [128, L], BF16, kind="ExternalOutput").ap()
        io["dbg_brep"] = nc.dram_tensor("dbg_brep", [128, L], BF16, kind="ExternalOutput").ap()
        io["dbg_crep"] = nc.dram_tensor("dbg_crep", [128, L], BF16, kind="ExternalOutput").ap()
        io["dbg_del0"] = nc.dram_tensor("dbg_del0", [128, L], BF16, kind="ExternalOutput").ap()
        io["dbg_xc0"] = nc.dram_tensor("dbg_xc0", [128, L], BF16, kind="ExternalOutput").ap()
    with tile.TileContext(nc) as tc:
        with ExitStack() as ctx:
            build_kernel(ctx, tc, io)
    nc.compile()
    _NC_CACHE["nc"] = nc
    return nc


def make_in_maps(x, W_in, conv_w, conv_b, x_proj_w, dt_proj_w, dt_proj_b, A_logs,
                 Ds, W_out):
    import ml_dtypes

    f = lambda a: np.ascontiguousarray(np.asarray(a, dtype=np.float32))
    a_col, rt64, ident = _host_constants(A_logs)
    xpw = f(x_proj_w)
    common = {
        "w_inT": np.ascontiguousarray(f(W_in).T), "conv_b": f(conv_b),
        "xpb_wT": np.ascontiguousarray(np.repeat(xpw[RK : RK + N], 8, axis=0).T.astype(ml_dtypes.bfloat16)),
        "xpc_wT": np.ascontiguousarray(np.repeat(xpw[RK + N : RK + 2 * N], 8, axis=0).T.astype(ml_dtypes.bfloat16)),
        "dtw_fullT": np.ascontiguousarray((f(dt_proj_w) @ xpw[:RK]).T.astype(ml_dtypes.bfloat16)),
        "dt_proj_b": f(dt_proj_b),
        "d2": f(Ds) * 2.0, "w_outT": np.ascontiguousarray(f(W_out).T),
        "a_col": a_col,
        "rt64": np.ascontiguousarray(rt64.astype(ml_dtypes.bfloat16)),
        "ident": ident,
    }
    x = f(x)
    cw = f(conv_w)
    cw_t = np.ascontiguousarray(cw.transpose(0, 1, 3, 2))
    in_maps = []
    for c in range(8):
        b, lay = c // 2, c % 2
        xv = x[b] if lay == 0 else np.ascontiguousarray(x[b].transpose(1, 0, 2))
        cwl = cw if lay == 0 else cw_t
        in_maps.append(
            {**common, "x": xv.reshape(L, CM),
             "conv_wT": np.ascontiguousarray(cwl.transpose(2, 3, 1, 0))}
        )
    return in_maps


def assemble(parts):
    out = np.zeros((B, L, CM), np.float32)
    for c in range(8):
        b, lay = c // 2, c % 2
        p = parts[c].T  # device emits [96, L]
        if lay:
            p = p.reshape(W, H, CM).transpose(1, 0, 2).reshape(L, CM)
        out[b] += p
    return out.reshape(B, H, W, CM)


def kernel(**inputs):
    from concourse.bass_utils import run_bass_kernel_spmd

    nc = _get_nc()
    in_maps = make_in_maps(**inputs)
    res = run_bass_kernel_spmd(nc, in_maps, list(range(8)))
    return assemble([res.results[c]["out"] for c in range(8)])


# revision 35
# speedup vs baseline: 1.6819x; 1.1048x over previous
"""BiMamba2D (VMamba-style 4-direction selective scan) Trainium2 Bass kernel.

Sharding: 8 cores = 4 batches x 2 scan layouts (hw / wh); each core runs both
time directions of its layout and emits a partial (96, L) output; the host
sums partials.

Scan-phase design (v2):
  * State layout is n-MAJOR: partition p of a d-block holds (state n = p//8,
    channel c = p%8).  This makes the 8->128 partition replication of
    delta / delta*u a chain of 5 partition-contiguous SBUF->SBUF DMAs
    (doubling), entirely off the compute engines.
  * All elementwise work lives on the DVE in bf16 2x mode; GpSimd is idle
    (measured: gpsimd ops and DVE scans mutually block on the shared SBUF
    port pair, nearly serializing the two engines).
  * Scans are single full-L [128, 4096] tensor_tensor_scan ops (48 total):
    ~12% cheaper per element than chunked scans, no h chaining, and dA/dBu
    are computed once and read by both the forward scan and the reversed-AP
    backward scan (no recompute, no DRAM spill).
  * y = sum_n C*h accumulates via 0/1 matmuls into 8 PSUM banks (one per
    time chunk); both directions of all 8 d-blocks of a group accumulate
    into the same banks, so writeback is one PSUM->SBUF copy per chunk.
"""

import os
import sys
from contextlib import ExitStack

import numpy as np

for _p in ("/opt/trn_rl_repo",):
    if _p not in sys.path and os.path.isdir(_p):
        sys.path.append(_p)

import concourse.bass as bass
import concourse.tile as tile
from concourse import bacc, mybir

F32 = mybir.dt.float32
F32R = mybir.dt.float32r
BF16 = mybir.dt.bfloat16
AL = mybir.AluOpType
AF = mybir.ActivationFunctionType

DEBUG = os.environ.get("KDBG", "0") not in ("0", "5")
DEBUG_KEEP = os.environ.get("KDBG") == "5"
DEBUG_J = os.environ.get("KDBG") in ("1", "3")   # per-j dumps
DEBUG_P4 = os.environ.get("KDBG") in ("1", "2")  # end-of-phase-4 dumps
DBG_G = int(os.environ.get("KDBG_G", "0"))
DBG_J_IDX = int(os.environ.get("KDBG_JIDX", "0"))


def _r(ap):
    """View an fp32 AP as float32r: single-pass PE matmul at tf32-like
    precision, plenty for this tolerance."""
    return ap.bitcast(F32R)

# Problem constants
B, H, W, CM = 4, 64, 64, 96
L = H * W  # 4096
D = 192  # d_inner
N = 16  # d_state
RK = 6  # dt_rank
TC = 512  # time-chunk (PSUM bank size)
NCH = L // TC  # 8
NG = 3  # groups of 64 channels
GDB = 8  # d-blocks per group
HS = [128, 64]  # d_inner row split
HOF = [0, 128]  # absolute channel offset per half
# group -> (half index, row offset within half)
GMAP = [(0, 0), (0, 64), (1, 0)]
WP = W + 2  # padded row stride for conv


def build_kernel(ctx: ExitStack, tc: "tile.TileContext", io: dict):
    nc = tc.nc

    # ---------------- weight / constant loads ----------------
    wpool = ctx.enter_context(tc.tile_pool(name="wpool", bufs=1))

    w_int = wpool.tile([96, 384], F32R, name="w_int")
    nc.sync.dma_start(w_int[:], io["w_inT"])

    # x first: everything in the prologue is gated on it; the ~55 weight DMAs
    # behind it would otherwise delay the first in-proj matmul by ~45us.
    # ---------------- persistent big buffers ----------------
    ppool = ctx.enter_context(tc.tile_pool(name="persist", bufs=1))
    xc = [ppool.tile([HS[hh], L], BF16, name=f"xc{hh}") for hh in range(2)]
    y_sb = [ppool.tile([HS[hh], L], BF16, name=f"y{hh}") for hh in range(2)]
    b_rep = ppool.tile([128, L], BF16, name="b_rep")
    c_rep = ppool.tile([128, L], BF16, name="c_rep")
    # softplus(dt) for all inner channels, precomputed once (phase 3.5)
    del_sb = [ppool.tile([HS[hh], L], BF16, name=f"del{hh}") for hh in range(2)]
    # silu(z) gate, computed in phase 1 while xT is resident
    z_act = [ppool.tile([HS[hh], L], BF16, name=f"z_act{hh}") for hh in range(2)]

    p12 = ExitStack()  # closed at end of phase 2
    p1big = p12.enter_context(tc.tile_pool(name="p1big", bufs=1))
    xT = p1big.tile([96, L], F32R, name="xT")
    nc.sync.dma_start(xT[:], io["x"][:])
    # conv weights next (needed ~15us in)
    cwpool = p12.enter_context(tc.tile_pool(name="cwpool", bufs=1))
    cw = {}
    for ih in range(2):
        for oh in range(2):
            for kh in range(3):
                for kw in range(3):
                    t = cwpool.tile([HS[ih], HS[oh]], BF16, name=f"cw{ih}{oh}{kh}{kw}")
                    src = io["conv_wT"][
                        kh, kw,
                        HOF[ih] : HOF[ih] + HS[ih],
                        HOF[oh] : HOF[oh] + HS[oh],
                    ]
                    nc.sync.dma_start(t[:], src)
                    cw[(ih, oh, kh, kw)] = t

    # PE warm-up: ~6us of dummy matmuls so the HAM clock gate opens (K=8/8,
    # 2.4 GHz) before the real prologue stream instead of ~70us into it.
    # Output bank is never read.
    with tc.tile_pool(name="warmps", bufs=1, space="PSUM") as warmps:
        ps_w = warmps.tile([128, 384], F32, name="ps_w")
        for _ in range(45):
            nc.tensor.matmul(ps_w[:], w_int[:, 0:128], w_int[:], start=True, stop=True)
        del ps_w

    # B/C projections with 16->128 n-major row replication folded in
    # (host-tiled), and the dt projection folded through x_proj.
    xpb_t, xpc_t, dtw_t = [], [], []
    for hh in range(2):
        hsl = slice(HOF[hh], HOF[hh] + HS[hh])
        t = wpool.tile([HS[hh], 128], BF16, name=f"xpb_t{hh}")
        nc.sync.dma_start(t[:], io["xpb_wT"][hsl, :])
        xpb_t.append(t)
        t = wpool.tile([HS[hh], 128], BF16, name=f"xpc_t{hh}")
        nc.sync.dma_start(t[:], io["xpc_wT"][hsl, :])
        xpc_t.append(t)
        t = wpool.tile([HS[hh], 192], BF16, name=f"dtw_t{hh}")
        nc.sync.dma_start(t[:], io["dtw_fullT"][hsl, :])
        dtw_t.append(t)

    wout_t = []
    for hh in range(2):
        t = wpool.tile([HS[hh], 96], F32R, name=f"wout_t{hh}")
        nc.sync.dma_start(t[:], io["w_outT"][HOF[hh] : HOF[hh] + HS[hh], :])
        wout_t.append(t)

    def vec_col(name):
        tiles = []
        for hh in range(2):
            t = wpool.tile([HS[hh], 1], F32, name=f"{name}{hh}")
            nc.sync.dma_start(
                t[:],
                io[name][HOF[hh] : HOF[hh] + HS[hh]].rearrange("(p one) -> p one", one=1),
            )
            tiles.append(t)
        return tiles

    dtb = vec_col("dt_proj_b")
    convb = vec_col("conv_b")
    d2 = vec_col("d2")

    a_col = wpool.tile([128, 1], F32, name="a_col")
    nc.sync.dma_start(a_col[:], io["a_col"][:])
    rt64 = []  # [j]: [128, 64] n-contraction lhsT: 1 iff d64 == j*8 + p%8
    for j in range(GDB):
        t = wpool.tile([128, 64], BF16, name=f"rt64_{j}")
        nc.sync.dma_start(t[:], io["rt64"][j])
        rt64.append(t)

    # ================= phase 1: input projection =================
    if True:
        xp_pad = [
            p1big.tile([HS[hh], (H + 2) * WP], BF16, name=f"xp_pad{hh}")
            for hh in range(2)
        ]
        for hh in range(2):
            nc.gpsimd.memset(xp_pad[hh][:], 0.0)

        with (
            tc.tile_pool(name="p1ps", bufs=2, space="PSUM") as p1ps,
        ):
            for ch in range(NCH):
                tsl = slice(ch * TC, (ch + 1) * TC)
                for oh in range(2):
                    ps = p1ps.tile([HS[oh], TC], F32, tag=f"ps_ip{oh}")
                    nc.tensor.matmul(
                        ps[:],
                        _r(w_int[:, HOF[oh] : HOF[oh] + HS[oh]]),
                        xT[:, tsl],
                        start=True,
                        stop=True,
                    )
                    # write into padded conv buffer rows [ch*8+1..ch*8+8], cols 1..64
                    dst = (
                        xp_pad[oh][:]
                        .rearrange("p (h w) -> p h w", w=WP)[
                            :, ch * 8 + 1 : ch * 8 + 9, 1 : W + 1
                        ]
                    )
                    nc.vector.tensor_copy(dst, ps[:])
                # z projection + silu while xT is resident
                for hh in range(2):
                    ps_z = p1ps.tile([HS[hh], TC], F32, tag=f"ps_ip{hh}")
                    nc.tensor.matmul(
                        ps_z[:],
                        _r(w_int[:, 192 + HOF[hh] : 192 + HOF[hh] + HS[hh]]),
                        xT[:, tsl],
                        start=True,
                        stop=True,
                    )
                    nc.scalar.activation(z_act[hh][:, tsl], ps_z[:], AF.Silu)

        # ================= phase 2: 3x3 conv + bias + silu =================
        TCC = 512
        with tc.tile_pool(name="p2ps", bufs=2, space="PSUM") as p2ps:
            for ch in range(L // TCC):
                tsl = slice(ch * TCC, (ch + 1) * TCC)
                for oh in range(2):
                    ps = p2ps.tile([HS[oh], TCC], F32, tag=f"ps_cv{oh}")
                    first = True
                    for ih in range(2):
                        for kh in range(3):
                            for kw in range(3):
                                rhs = (
                                    xp_pad[ih][:]
                                    .rearrange("p (h w) -> p h w", w=WP)[
                                        :, ch * 8 + kh : ch * 8 + kh + 8, kw : kw + W
                                    ]
                                )
                                last = ih == 1 and kh == 2 and kw == 2
                                nc.tensor.matmul(
                                    ps[:],
                                    cw[(ih, oh, kh, kw)][:],
                                    rhs,
                                    start=first,
                                    stop=last,
                                )
                                first = False
                    nc.scalar.activation(
                        xc[oh][:, tsl], ps[:], AF.Silu, bias=convb[oh][:, 0:1]
                    )
                # B/C projections for this chunk right away: keeps the PE
                # stream dense across the old phase-2/3 boundary.
                for half in range(TCC // TC):
                    psl = slice(ch * TCC + half * TC, ch * TCC + (half + 1) * TC)
                    ps_b = p2ps.tile([128, TC], F32, tag="ps_bc")
                    nc.tensor.matmul(ps_b[:], xpb_t[0][:], xc[0][:, psl], start=True, stop=False)
                    nc.tensor.matmul(ps_b[:], xpb_t[1][:], xc[1][:, psl], start=False, stop=True)
                    nc.vector.tensor_copy(b_rep[:, psl], ps_b[:])
                    ps_c = p2ps.tile([128, TC], F32, tag="ps_bc")
                    nc.tensor.matmul(ps_c[:], xpc_t[0][:], xc[0][:, psl], start=True, stop=False)
                    nc.tensor.matmul(ps_c[:], xpc_t[1][:], xc[1][:, psl], start=False, stop=True)
                    nc.vector.tensor_copy(c_rep[:, psl], ps_c[:])

    p12.close()

    # ============ phase 3: B/C projection (replication folded in) ============
    # re-warm the PE clock gate (it re-throttles during the phase-2 tail)
    with tc.tile_pool(name="warmps2", bufs=1, space="PSUM") as warmps2:
        ps_w2 = warmps2.tile([128, 384], F32, name="ps_w2")
        for _ in range(16):
            nc.tensor.matmul(ps_w2[:], w_int[:, 0:128], w_int[:], start=True, stop=True)
        del ps_w2
    with tc.tile_pool(name="p3ps", bufs=2, space="PSUM") as p3ps:
        # ---- phase 3.5: delta prologue: del = softplus(dtw @ xc + dtb) ----
        # hh-outer so half 0 finishes first and group 0's spill/replication
        # chain can launch while half 1 is still projecting.
        for hh in range(2):
            for ch in range(NCH):
                tsl = slice(ch * TC, (ch + 1) * TC)
                osl = slice(HOF[hh], HOF[hh] + HS[hh])
                ps = p3ps.tile([HS[hh], TC], F32, tag=f"ps35{hh}")
                nc.tensor.matmul(
                    ps[:], dtw_t[0][:, osl], xc[0][:, tsl],
                    start=True, stop=False,
                )
                nc.tensor.matmul(
                    ps[:], dtw_t[1][:, osl], xc[1][:, tsl],
                    start=False, stop=True,
                )
                nc.scalar.activation(
                    del_sb[hh][:, tsl], ps[:], AF.Exp, bias=dtb[hh][:, 0:1]
                )
        for hh in range(2):
            for ch in range(NCH):
                tsl = slice(ch * TC, (ch + 1) * TC)
                nc.scalar.activation(
                    del_sb[hh][:, tsl], del_sb[hh][:, tsl], AF.Ln, bias=1.0
                )

    # ================= phase 4: selective scan (fwd + rev) =================
    # 8 -> 128 partition replication (n-major): spill the group rows to DRAM
    # once, then one DMA per d-block reads them back through a broadcast AP.
    # (Chained same-tile SBUF->SBUF DMAs race on hardware; DRAM round-trip
    # DMA->DMA dependencies are reliable.)
    with (
        tc.tile_pool(name="spillp", bufs=2, space="DRAM") as spillp,
        tc.tile_pool(name="dreppool", bufs=2) as dreppool,
        tc.tile_pool(name="dapool", bufs=2) as dapool,
        tc.tile_pool(name="ureppool", bufs=2) as ureppool,
        tc.tile_pool(name="dbupool", bufs=2) as dbupool,
        tc.tile_pool(name="hpool", bufs=2) as hpool,
        tc.tile_pool(name="opool", bufs=2) as opool,
        tc.tile_pool(name="ducpool", bufs=1) as ducpool,
        tc.tile_pool(name="wbpool", bufs=2) as wbpool,
        tc.tile_pool(name="scpsy", bufs=1, space="PSUM") as scpsy,
    ):
        # du = delta * conv-act for all groups up front + DRAM spills, so the
        # replication DMA chain never stalls a group boundary.
        du_c = [ducpool.tile([HS[hh], L], BF16, name=f"du_c{hh}") for hh in range(2)]
        del_sps, du_sps = [], []
        for g in range(NG):
            hh, gr0 = GMAP[g]
            gp = slice(gr0, gr0 + 64)
            nc.vector.tensor_tensor(
                du_c[hh][gp, :], del_sb[hh][gp, :], xc[hh][gp, :], AL.mult
            )
            del_sp = spillp.tile([64, L], BF16, tag=f"del_sp{g}", name=f"del_sp{g}")
            nc.sync.dma_start(del_sp[:], del_sb[hh][gp, :])
            del_sps.append(del_sp)
            du_sp = spillp.tile([64, L], BF16, tag=f"du_sp{g}", name=f"du_sp{g}")
            nc.sync.dma_start(du_sp[:], du_c[hh][gp, :])
            du_sps.append(du_sp)

        for g in range(NG):
            hh, gr0 = GMAP[g]
            gp = slice(gr0, gr0 + 64)
            del_sp, du_sp = del_sps[g], du_sps[g]
            psY = [
                scpsy.tile([128, TC], F32, tag=f"psY{c}", name=f"psY{c}")
                for c in range(NCH)
            ]

            for j in range(GDB):
                rsl = slice(gr0 + j * 8, gr0 + j * 8 + 8)
                jsl = slice(j * 8, (j + 1) * 8)
                drep = dreppool.tile([128, L], BF16, tag="drep")
                nc.sync.dma_start(
                    drep[:], del_sp[jsl, :].unsqueeze(0).broadcast_to([16, 8, L])
                )
                dA = dapool.tile([128, L], BF16, tag="dA")
                nc.scalar.activation(dA[:], drep[:], AF.Exp, scale=a_col[:, 0:1])

                urep = ureppool.tile([128, L], BF16, tag="urep")
                nc.sync.dma_start(
                    urep[:], du_sp[jsl, :].unsqueeze(0).broadcast_to([16, 8, L])
                )
                dBu = dbupool.tile([128, L], BF16, tag="dBu")
                nc.vector.tensor_tensor(dBu[:], urep[:], b_rep[:], AL.mult)

                if os.environ.get("KDBG") == "5" and g == DBG_G and j == DBG_J_IDX:
                    kp1 = wpool.tile([128, 512], BF16, name="kp1")
                    kp2 = wpool.tile([128, 512], BF16, name="kp2")
                    nc.vector.tensor_copy(kp1[:], drep[:, 0:512])
                    nc.vector.tensor_copy(kp2[:], urep[:, 0:512])
                    nc.sync.dma_start(io["dbg_drep"][:, 0:512], kp1[:])
                    nc.sync.dma_start(io["dbg_urep"][:, 0:512], kp2[:])
                h_f = hpool.tile([128, L], BF16, tag="h")
                nc.vector.tensor_tensor_scan(h_f[:], dA[:], dBu[:], 0.0, AL.mult, AL.add)
                o_f = opool.tile([128, L], BF16, tag="o")
                nc.vector.tensor_tensor(o_f[:], h_f[:], c_rep[:], AL.mult)
                for c in range(NCH):
                    csl = slice(c * TC, (c + 1) * TC)
                    nc.tensor.matmul(
                        psY[c][0:64, :], rt64[j][:], o_f[:, csl],
                        start=(j == 0), stop=False,
                    )

                h_r = hpool.tile([128, L], BF16, tag="h")
                nc.vector.tensor_tensor_scan(
                    h_r[:], dA[:, ::-1], dBu[:, ::-1], 0.0, AL.mult, AL.add
                )
                # time-corrected: o_r[t] = h_r[L-1-t] * C[t]
                o_r = opool.tile([128, L], BF16, tag="o")
                nc.vector.tensor_tensor(o_r[:], h_r[:, ::-1], c_rep[:], AL.mult)
                if DEBUG_J and g == DBG_G and j == DBG_J_IDX:
                    for nm, t in [("dbg_drep", drep), ("dbg_dA", dA),
                                  ("dbg_urep", urep), ("dbg_dBu", dBu),
                                  ("dbg_hf", h_f), ("dbg_of", o_f),
                                  ("dbg_hr", h_r), ("dbg_or", o_r)]:
                        nc.sync.dma_start(io[nm][:], t[:])
                for c in range(NCH):
                    csl = slice(c * TC, (c + 1) * TC)
                    nc.tensor.matmul(
                        psY[c][0:64, :], rt64[j][:], o_r[:, csl],
                        start=False, stop=(j == GDB - 1),
                    )

            # ---- writeback: one PSUM->SBUF copy per chunk ----
            # last group: copies on V (idle there, and keeps the tail's
            # critical path on one engine); mid-window groups: on S (V is
            # saturated with scans then).
            for c in range(NCH):
                csl = slice(c * TC, (c + 1) * TC)
                if gr0 == 0:
                    if g == NG - 1:
                        nc.vector.tensor_copy(y_sb[hh][0:64, csl], psY[c][0:64, :])
                    else:
                        nc.scalar.copy(y_sb[hh][0:64, csl], psY[c][0:64, :])
                else:
                    # engines cannot shift partitions; bounce via SBUF + DMA
                    wt = wbpool.tile([128, TC], BF16, tag="wt")
                    nc.scalar.copy(wt[0:64, :], psY[c][0:64, :])
                    nc.sync.dma_start(y_sb[hh][64:128, csl], wt[0:64, :])

    if DEBUG_P4:
        nc.sync.dma_start(io["dbg_ysb0"][:], y_sb[0][:])
        nc.sync.dma_start(io["dbg_brep"][:], b_rep[:])
        nc.sync.dma_start(io["dbg_crep"][:], c_rep[:])
        nc.sync.dma_start(io["dbg_del0"][:], del_sb[0][:])
        nc.sync.dma_start(io["dbg_xc0"][:], xc[0][:])

    # phase-5's scoped pools reuse the scan-phase SBUF/PSUM addresses; fence
    # so nothing in phase 5 can clobber tiles still being read.
    tc.strict_bb_all_engine_barrier()

    # ======== phase 5: z-gate, D*u, out-projection (per chunk, DMA out) ========
    with (
        tc.tile_pool(name="p6ps", bufs=4, space="PSUM") as p6ps,
        tc.tile_pool(name="p6sb", bufs=4) as p6sb,
    ):
        for ch in range(NCH):
            tsl = slice(ch * TC, (ch + 1) * TC)
            yg = []
            for hh in range(2):
                yf = p6sb.tile([HS[hh], TC], F32, tag=f"yf{hh}")
                nc.vector.scalar_tensor_tensor(
                    yf[:], xc[hh][:, tsl], d2[hh][:, 0:1], y_sb[hh][:, tsl],
                    AL.mult, AL.add,
                )
                g = p6sb.tile([HS[hh], TC], F32, tag=f"yg{hh}")
                nc.vector.tensor_tensor(_r(g[:]), yf[:], z_act[hh][:, tsl], AL.mult)
                yg.append(g)

            ps_o = p6ps.tile([96, TC], F32, tag="ps_o")
            nc.tensor.matmul(ps_o[:], _r(wout_t[0][:]), _r(yg[0][:]), start=True, stop=False)
            nc.tensor.matmul(ps_o[:], _r(wout_t[1][:]), _r(yg[1][:]), start=False, stop=True)
            out_c = p6sb.tile([96, TC], F32, tag="out_c")
            nc.scalar.copy(out_c[:], ps_o[:])
            nc.sync.dma_start(io["out"][:, tsl], out_c[:])
